# revision 27
# baseline (speedup 1.0000x reference)
"""Trainium2 Bass kernel for DiffAttention (nn_DiffAttention_49847390437777).

Contract: kernel(**full_inputs) -> full output [2, 2048, 8, 256] fp32.

Sharding (8 cores): core c handles batch b = c//4 and global query-head pairs
{2*(c%4), 2*(c%4)+1}.  Diff-attention couples only adjacent head pairs, which
stay co-located.  lambda scalars are computed on host and shipped as a tiny
replicated tensor; subln_weight is applied on host after the gather.

Device algorithm per core (4 heads = 2 pairs, seq 2048, head_dim 128),
all-bf16 matmuls:
  - scores transposed: S^T[k, q] = kT_blk.T @ qT_blk, causal blocks only.
  - softmax without max-subtraction; rowsum fused into the PV matmul via a
    ones-column appended to V:  O = P^T.T @ [v1|v2|1].
  - P exp'd on ACT (PSUM->SBUF) bf16; causal diagonal 128x128 blocks masked
    with a precomputed triangular tile on GPSIMD (keeps DVE free for the
    epilogue chains during score-heavy phases).
  - epilogue on UNNORMALIZED attention, rescaled by r2/(lam*r1):
      D   = (O1/(lam*r1))*r2 - O2          (r = rowsums from the ones col)
      out = D / sqrt(ssq(D)/(256*S_FOLD^2) + (EPS/(lam*S_FOLD))^2'ish*r2^2)
    t_pre = O1/(lam*r1) is computed DURING par1's PV matmuls (off the
    critical path); after par1 only a short chain remains: D (one fused
    scalar_tensor_tensor), ssq (fused square+accum into the rms statistic),
    bias.  S_FOLD is folded into the denominator scaling host-side; rms runs
    as exp(-0.5*ln(x)) on ACT (DVE pow fails ISA codegen; ACT Rsqrt banned
    for accuracy).
  - pipeline: per superblock, score groups alternate par0/par1 (so the first
    PV unit's exp deps complete early), then the previous superblock's four
    PV units; flattened across the two pairs so pair1's scores overlap
    pair0's last PV.  Warm-up matmuls on the triangular const keep the PE
    HAM clock-gate warm through the input-DMA window.  Diagonal-group exps
    are fused per 2-block group (the stale half-row is provably never read).
  - finalize is batched per superblock except qb3 of pair1 (per-j, to
    shorten the post-matmul tail; its last outputs issue from the idle sync
    queue).  Output stored bf16 (error ~0.2% of element value, well inside
    the 2e-2 gate), halving output DMA.
"""

import math
import os

import numpy as np
import ml_dtypes

HEAD_DIM = 128
N_HEADS = 16
LAYER_IDX = 12
LAMBDA_INIT = 0.8 - 0.6 * math.exp(-0.3 * (LAYER_IDX - 1))
EPS = 1e-5
SCALE = 1.0 / math.sqrt(HEAD_DIM)
S_FOLD = 1.0 - LAMBDA_INIT
DEN_SCALE = 1.0 / (256.0 * S_FOLD * S_FOLD)

B = 2
S = 2048
NB = S // 128
QB = S // 512
N_CORES = 8

bf16 = ml_dtypes.bfloat16

_CACHE = {}
last_results = None


def build_nc():
    import concourse.bass as bass
    import concourse.mybir as mybir
    import concourse.bacc as bacc
    import concourse.tile as tile
    from concourse.masks import make_upper_triangular
    from contextlib import ExitStack

    f32 = mybir.dt.float32
    b16 = mybir.dt.bfloat16
    AF = mybir.ActivationFunctionType
    ALU = mybir.AluOpType

    nc = bacc.Bacc("TRN2", target_bir_lowering=False, debug=False)

    kq = nc.dram_tensor("kq", [2, 2, 2, 128, S], b16, kind="ExternalInput")
    vxb = nc.dram_tensor("vxb", [2, 128, NB, 257], b16, kind="ExternalInput")
    # misc col0 = lam_full, col1 = EPS/lam_full^2
    misc = nc.dram_tensor("misc", [128, 2], f32, kind="ExternalInput")
    o = nc.dram_tensor("o", [2, NB, 128, 256], b16, kind="ExternalOutput")

    with tile.TileContext(nc) as tc:
        with ExitStack() as ctx:
            ec = ctx.enter_context
            const = ec(tc.tile_pool(name="const", bufs=1))
            kqpool = ec(tc.tile_pool(name="kqpool", bufs=2))
            vpool = ec(tc.tile_pool(name="vpool", bufs=2))
            ppool = ec(tc.tile_pool(name="ppool", bufs=4))
            dpool = ec(tc.tile_pool(name="dpool", bufs=8))
            stat = ec(tc.tile_pool(name="stat", bufs=8))
            tmp = ec(tc.tile_pool(name="tmp", bufs=4))
            opool = ec(tc.tile_pool(name="opool", bufs=4))
            spsum = ec(tc.tile_pool(name="spsum", bufs=2, space="PSUM"))
            opsum = ec(tc.tile_pool(name="opsum", bufs=4, space="PSUM"))

            misc_t = const.tile([128, 2], f32)
            lam_ap = misc_t[:, 0:1]
            c_ap = misc_t[:, 1:2]
            tri16 = const.tile([128, 128], b16)

            kqt, vxt = {}, {}
            P = {}          # (pair, qb, par) -> tile
            ssq4b, bias4b = {}, {}   # (pair, qb) -> [128,4] stat tiles
            dts = {}        # (pair, qb, j) -> D tile

            def emit_pair_dmas(p):
                kq_t = kqpool.tile([128, 2, 2, S], b16, tag="kq", name="kq_t")
                # first chunk issues go out on parallel queues; pair0's
                # head chunks are split small so the first matmul starts early
                if p == 0:
                    nc.gpsimd.dma_start(kq_t[:, 0, 0, 0:128],
                                        kq[p, 0, 0][:, 0:128])
                    nc.sync.dma_start(kq_t[:, 0, 1, 0:256],
                                      kq[p, 0, 1][:, 0:256])
                    nc.gpsimd.dma_start(kq_t[:, 0, 0, 128:512],
                                        kq[p, 0, 0][:, 128:512])
                    nc.sync.dma_start(kq_t[:, 0, 1, 256:512],
                                      kq[p, 0, 1][:, 256:512])
                else:
                    nc.gpsimd.dma_start(kq_t[:, 0, 0, 0:512],
                                        kq[p, 0, 0][:, 0:512])
                    nc.sync.dma_start(kq_t[:, 0, 1, 0:512],
                                      kq[p, 0, 1][:, 0:512])
                for i in range(2):
                    nc.sync.dma_start(kq_t[:, 1, i, 0:512],
                                      kq[p, 1, i][:, 0:512])
                vx_t = vpool.tile([128, NB, 257], b16, tag="vx", name="vx_t")
                nc.sync.dma_start(vx_t[:, 0:4, :], vxb[p][:, 0:4, :])
                for par in range(2):
                    for i in range(2):
                        nc.sync.dma_start(kq_t[:, par, i, 512:S],
                                          kq[p, par, i][:, 512:S])
                nc.sync.dma_start(vx_t[:, 4:NB, :], vxb[p][:, 4:NB, :])
                kqt[p], vxt[p] = kq_t, vx_t

            def emit_consts():
                nc.gpsimd.dma_start(misc_t[:], misc[:])
                make_upper_triangular(nc, tri16[:], val=1.0, diag=True)

            def emit_score_group(p, q, par, g):
                if (p, q, par) not in P:
                    P[(p, q, par)] = ppool.tile([128, NB, 512], b16,
                                                tag="pt", name="pt")
                p1 = P[(p, q, par)]
                kt = kqt[p][:, par, 0, :]
                qt = kqt[p][:, par, 1, :]
                q0 = q * 512
                sp = spsum.tile([128, 2, 512], f32, tag="sp")
                for t in range(2):
                    kb = 2 * g + t
                    qoff = 0 if kb < 4 * q else (kb - 4 * q) * 128
                    if p == 0 and q == 0 and par == 0 and kb == 0:
                        # split: depends only on the small head DMA chunks
                        nc.tensor.matmul(sp[:, 0, 0:256], kt[:, 0:128],
                                         qt[:, 0:256], start=True, stop=True)
                        nc.tensor.matmul(sp[:, 0, 256:512], kt[:, 0:128],
                                         qt[:, 256:512], start=True, stop=True)
                        continue
                    nc.tensor.matmul(
                        sp[:, t, qoff:512],
                        kt[:, kb * 128:(kb + 1) * 128],
                        qt[:, q0 + qoff:q0 + 512],
                        start=True, stop=True,
                    )
                # one exp per group: for diagonal groups the second row's
                # [qoff0:qoff0+128) region is stale PSUM -> exp'd garbage, but
                # that P region is provably never read by any PV matmul.
                # qb0 exps stay per-block so the first PV unit unblocks early.
                if q == 0:
                    for t in range(2):
                        kb = 2 * g + t
                        qoff = kb * 128
                        nc.scalar.activation(p1[:, kb, qoff:512],
                                             sp[:, t, qoff:512],
                                             AF.Exp, scale=SCALE)
                else:
                    qoff0 = 0 if 2 * g < 4 * q else (2 * g - 4 * q) * 128
                    nc.scalar.activation(p1[:, 2 * g:2 * g + 2, qoff0:512],
                                         sp[:, :, qoff0:512],
                                         AF.Exp, scale=SCALE)
                for t in range(2):
                    kb = 2 * g + t
                    if kb >= 4 * q:
                        qoff = (kb - 4 * q) * 128
                        nc.gpsimd.tensor_mul(
                            p1[:, kb, qoff:qoff + 128],
                            p1[:, kb, qoff:qoff + 128], tri16[:])

            def emit_finalize_j(p, q, j, ssq1, bias1, dt, last=False):
                den = stat.tile([128, 1], f32, tag="den1")
                nc.vector.scalar_tensor_tensor(
                    den[:], ssq1[:], DEN_SCALE, bias1[:], ALU.mult, ALU.add)
                lnm = stat.tile([128, 1], f32, tag="lnm1")
                nc.scalar.activation(lnm[:], den[:], AF.Ln)
                rms = stat.tile([128, 1], f32, tag="rms1")
                nc.scalar.activation(rms[:], lnm[:], AF.Exp, scale=-0.5)
                ot = opool.tile([128, 256], b16, tag="ot")
                nc.vector.tensor_scalar_mul(ot[:], dt[:], rms[:])
                eng = nc.sync if last else nc.gpsimd
                eng.dma_start(o[p, 4 * q + j], ot[:])

            def emit_finalize_batch(p, q):
                ssq4, bias4 = ssq4b[(p, q)], bias4b[(p, q)]
                den4 = stat.tile([128, 4], f32, tag="den")
                nc.vector.scalar_tensor_tensor(
                    den4[:], ssq4[:], DEN_SCALE, bias4[:], ALU.mult, ALU.add)
                lnm = stat.tile([128, 4], f32, tag="lnm")
                nc.scalar.activation(lnm[:], den4[:], AF.Ln)
                rmst = stat.tile([128, 4], f32, tag="rms")
                nc.scalar.activation(rmst[:], lnm[:], AF.Exp, scale=-0.5)
                for j in range(4):
                    ot = opool.tile([128, 256], b16, tag="ot")
                    nc.vector.tensor_scalar_mul(
                        ot[:], dts[(p, q, j)][:], rmst[:, j:j + 1])
                    nc.gpsimd.dma_start(o[p, 4 * q + j], ot[:])

            def pv_mms(p, q, j, par, op_t):
                jabs = 4 * q + j
                for kb in range(jabs + 1):
                    nc.tensor.matmul(
                        op_t[:, 0:257],
                        P[(p, q, par)][:, kb, j * 128:(j + 1) * 128],
                        vxt[p][:, kb, :],
                        start=(kb == 0), stop=(kb == jabs))

            def pv_tpre(op0):
                # t_pre = O1/(lam*r1): overlaps the par1 matmuls
                r1s = tmp.tile([128, 1], f32, tag="r1s")
                nc.vector.tensor_copy(r1s[:], op0[:, 256:257])
                lr = tmp.tile([128, 1], f32, tag="lr")
                nc.vector.tensor_scalar_mul(lr[:], r1s[:], lam_ap)
                w = tmp.tile([128, 1], f32, tag="w")
                nc.vector.reciprocal(w[:], lr[:])
                t_pre = tmp.tile([128, 256], f32, tag="tpre")
                nc.vector.tensor_scalar_mul(t_pre[:], op0[:, 0:256], w[:])
                return t_pre

            def emit_pv_unit(p, q, j, per_j, last=False):
                op = {}
                op[0] = opsum.tile([128, 257], f32, tag="op", name="op0")
                pv_mms(p, q, j, 0, op[0])
                t_pre = pv_tpre(op[0])
                op[1] = opsum.tile([128, 257], f32, tag="op", name="op1")
                pv_mms(p, q, j, 1, op[1])
                pv_post(p, q, j, op[1], t_pre, per_j, last)

            def pv_post(p, q, j, op1, t_pre, per_j, last):
                # short post-par1 chain
                r2s = tmp.tile([128, 1], f32, tag="r2s")
                nc.vector.tensor_copy(r2s[:], op1[:, 256:257])
                dt = dpool.tile([128, 256], b16, tag="dt", name="dt")
                dts[(p, q, j)] = dt
                nc.vector.scalar_tensor_tensor(
                    dt[:], t_pre[:], r2s[:], op1[:, 0:256],
                    ALU.mult, ALU.subtract)
                sqd = tmp.tile([128, 256], b16, tag="sqd")
                if per_j:
                    ssq1 = stat.tile([128, 1], f32, tag="ssq1")
                    bias1 = stat.tile([128, 1], f32, tag="bias1")
                    nc.vector.scalar_tensor_tensor(
                        sqd[:], dt[:], 1.0, dt[:], ALU.mult, ALU.mult,
                        accum_out=ssq1[:])
                    nc.vector.scalar_tensor_tensor(
                        bias1[:], r2s[:], c_ap, r2s[:], ALU.mult, ALU.mult)
                    emit_finalize_j(p, q, j, ssq1, bias1, dt, last=last)
                else:
                    if (p, q) not in ssq4b:
                        ssq4b[(p, q)] = stat.tile([128, 4], f32, tag="ssq4", name="ssq4")
                        bias4b[(p, q)] = stat.tile([128, 4], f32, tag="bias4", name="bias4")
                    nc.vector.scalar_tensor_tensor(
                        sqd[:], dt[:], 1.0, dt[:], ALU.mult, ALU.mult,
                        accum_out=ssq4b[(p, q)][:, j:j + 1])
                    nc.vector.scalar_tensor_tensor(
                        bias4b[(p, q)][:, j:j + 1], r2s[:], c_ap, r2s[:],
                        ALU.mult, ALU.mult)
                    if j == 3:
                        emit_finalize_batch(p, q)

            def pv(p, q, j):
                return ("pv", p, q, j)

            # stage construction: per stage, ordered list of score groups with
            # PV units interleaved so the in-order PE queue never starves.
            def stage_items(p, q):
                # sequential (v2-style) pipeline: all score groups of (p,q),
                # then the PV units of the previous superblock.  The Tile
                # scheduler's own reordering handles the fine-grained overlap
                # better than a hand-interleaved emission (measured).
                ngr = 2 * q + 2
                items = [("sc", p, q, par, g)
                         for g in range(ngr) for par in range(2)]
                if q == 0:
                    if p > 0:
                        items += [pv(p - 1, 3, j) for j in range(4)]
                else:
                    items += [pv(p, q - 1, j) for j in range(4)]
                return items

            emit_pair_dmas(0)
            emit_consts()

            for p in range(2):
                if p == 1:
                    emit_pair_dmas(1)
                for q in range(QB):
                    for it in stage_items(p, q):
                        if it[0] == "sc":
                            _, sp_, sq_, par, g = it
                            emit_score_group(sp_, sq_, par, g)
                        else:
                            _, up, uq, uj = it
                            emit_pv_unit(up, uq, uj,
                                         per_j=(up == 1 and uq == 3))
            # phantom: PV of pair1 qb3; the last two units are interleaved
            # so unit j3's t_pre chain sits ahead of unit j2's finalize in the
            # DVE queue (no head-of-line block past the final matmul).
            emit_pv_unit(1, 3, 0, per_j=True)
            emit_pv_unit(1, 3, 1, per_j=True)
            opA0 = opsum.tile([128, 257], f32, tag="op", name="opA0")
            pv_mms(1, 3, 2, 0, opA0)
            tpreA = pv_tpre(opA0)
            opA1 = opsum.tile([128, 257], f32, tag="op", name="opA1")
            pv_mms(1, 3, 2, 1, opA1)
            opB0 = opsum.tile([128, 257], f32, tag="op", name="opB0")
            pv_mms(1, 3, 3, 0, opB0)
            tpreB = pv_tpre(opB0)
            pv_post(1, 3, 2, opA1, tpreA, per_j=True, last=True)
            opB1 = opsum.tile([128, 257], f32, tag="op", name="opB1")
            pv_mms(1, 3, 3, 1, opB1)
            pv_post(1, 3, 3, opB1, tpreB, per_j=True, last=True)

    _orig_gat = bacc.get_activation_tables

    def _gat(arch):
        tabs = _orig_gat(arch)
        for name, fns in tabs.items():
            if name != "natural_log_exp_and_others":
                fns.discard(mybir.ActivationFunctionType.Exp)
                fns.discard(mybir.ActivationFunctionType.Ln)
        return tabs

    bacc.get_activation_tables = _gat
    try:
        nc.compile()
    finally:
        bacc.get_activation_tables = _orig_gat
    return nc


def _prep_core_inputs(q, k, v, lam_full):
    misc_ = np.empty((128, 2), np.float32)
    misc_[:, 0] = lam_full
    misc_[:, 1] = EPS / (lam_full * lam_full * S_FOLD * S_FOLD)
    in_maps = []
    for c in range(N_CORES):
        b = c // 4
        h0 = 4 * (c % 4)
        qs = q[b, :, h0:h0 + 4, :].transpose(1, 2, 0)
        ks = k[b, :, h0:h0 + 4, :].transpose(1, 2, 0)
        kq_ = np.empty((2, 2, 2, 128, S), bf16)
        for pair in range(2):
            for par in range(2):
                h = 2 * pair + par
                kq_[pair, par, 0] = ks[h].astype(bf16)
                kq_[pair, par, 1] = qs[h].astype(bf16)
        vx = np.empty((2, S, 257), np.float32)
        for pair in range(2):
            vx[pair, :, :128] = v[b, :, h0 + 2 * pair, :]
            vx[pair, :, 128:256] = v[b, :, h0 + 2 * pair + 1, :]
            vx[pair, :, 256] = 1.0
        vxb_ = np.ascontiguousarray(
            vx.reshape(2, NB, 128, 257).transpose(0, 2, 1, 3)).astype(bf16)
        in_maps.append({"kq": kq_, "vxb": vxb_, "misc": misc_})
    return in_maps


def kernel(q, k, v, lambda_q1, lambda_k1, lambda_q2, lambda_k2,
           subln_weight, attention_mask):
    global last_results
    from concourse.bass_utils import run_bass_kernel_spmd

    q = np.ascontiguousarray(np.asarray(q, np.float32))
    k = np.ascontiguousarray(np.asarray(k, np.float32))
    v = np.ascontiguousarray(np.asarray(v, np.float32))
    lam1 = np.exp(np.sum(np.asarray(lambda_q1, np.float32)
                         * np.asarray(lambda_k1, np.float32), dtype=np.float32))
    lam2 = np.exp(np.sum(np.asarray(lambda_q2, np.float32)
                         * np.asarray(lambda_k2, np.float32), dtype=np.float32))
    lam_full = np.float32(lam1 - lam2 + np.float32(LAMBDA_INIT))

    if "nc" not in _CACHE:
        _CACHE["nc"] = build_nc()
    nc = _CACHE["nc"]

    in_maps = _prep_core_inputs(q, k, v, lam_full)
    trace = bool(int(os.environ.get("KERNEL_TRACE", "0")))
    kw = {}
    if trace:
        kw = dict(trace=True, trace_cores=list(range(N_CORES)))
    res = run_bass_kernel_spmd(nc, in_maps, core_ids=list(range(N_CORES)), **kw)
    last_results = res

    out = np.empty((B, S, N_HEADS // 2, 256), np.float32)
    for c in range(N_CORES):
        b = c // 4
        gp = 2 * (c % 4)
        oc = res.results[c]["o"].astype(np.float32).reshape(2, S, 256)
        out[b, :, gp, :] = oc[0]
        out[b, :, gp + 1, :] = oc[1]
    out *= np.asarray(subln_weight, np.float32)[None, None, None, :]
    return out
